# revision 55
# baseline (speedup 1.0000x reference)
"""Trainium2 Bass/Tile kernel for the GatedNode2Edge op.

Computes, for emb (B,C,N), th12_* (E,C), th5_* (E,):
    t_k  = th12_k @ emb[b]                      (E,N)
    m_k  = max(t_k[:,i], t_k[:,j]) pairwise     (E,N,N)
    adj  = relu(2*m_1 + th5_1*I)
    gate = sigmoid(relu(2*m_2 + th5_2*I))
    out  = adj * gate                           (B,E,N,N)

Sharding: the 64 (b,e) channels are split 8-per-core across 8 NeuronCores.

Math restructuring (off-diagonal): with the 2x folded into the weights
(t' = 2t), and using that max() commutes with the monotone sigmoid and
relu(x) = max(x, 0), sigmoid(relu(x)) = max(sigmoid(x), 0.5):
    adj[i,j]  = max(t'1_j,          max(t'1_i, 0))
    gate[i,j] = max(sigmoid(t'2_j), max(sigmoid(t'2_i), 0.5))
so the REPLICATED (column-varying) operands need no clipping at all — the
clips live entirely in the per-partition scalars. One fused custom-DVE op
per output strip:
    out = maxx(Src0, C0) * maxx(Src1, C1)
Src0/Src1 = t'1 / sigmoid(t'2) rows replicated across partitions, built
by a replicated-weight matmul on the PE (lhsT[c, m] = 2*th12_k[ch, c] for
all m) directly from emb — no row-space phase at all. The sigmoid is
applied by ACT during the PSUM->SBUF drain (free). C0/C1 are per-partition
column slices. The custom op carries a hand-authored 2X_1PORT uop program
(two parallel max/max/mul chains over the packed bf16 pair via SRC_*_HI,
written through WR0_LO/WR0_HI).

The output matrix is SYMMETRIC (max is), so the device only computes the
36 upper-triangle [128,128] blocks of each channel's (N,N) matrix. Row
strip r (rows r*128..r*128+127, cols r*128..N) is paired with strip 7-r
so each pair packs to a constant 9*128=1152 columns: the o2 tile is
[128, 4, 1152] = 9 KiB/partition, stored with ONE dma_start per channel
(1.125 MiB, all 16 SDMA engines) on the otherwise-idle Sync queue —
dma_start triggers cost ~0.6-0.8us of issuing-sequencer time each, which
is why the stores are batched this coarsely. The host mirrors the lower
triangle during unsharding (pure data placement, like the bf16->f32 cast).

The true diagonal d1*d2 (d1 = relu(t'1+th5_1), d2 = sigmoid(relu(t'2+th5_2))
= max(sigmoid(t'2+th5_2), 0.5)) is computed on-device as one extra custom-op
call on a [128, 8, 8] f32 tile and exported as a second tiny output; the
host scatters it onto the diagonals.

Startup is load-receipt-bound: the emb high half rides one merged dma with
the column-weight matrix, channel 0's replication weights ride a dedicated
2.5 KiB load (DMA-write deps are tile-granular — one dma per head tile),
and channel 0 runs its narrow high-half B-strips while the low half is
still in flight (its v-copy runs on the then-idle DVE). The tail is
DMA-drain-bound: the final two channels are interleaved at strip-pair
granularity with contiguous 288 KiB pair stores on alternating queues, so
their combined 2.25 MiB spreads over both compute windows and the final
drain is one pair. (Finer strip-granular stores were tried and are slower:
odd non-contiguous shapes cut SDMA efficiency; so is a quarter-granularity
channel-0 head: it shortens the head but lengthens the DVE stream more.)

Everything on-device is bf16 (harness tolerance 2e-2 >> bf16 rounding); the
host converts back to f32. This halves HBM write traffic and halves matmul
LDWEIGHTS time.

Measured on TRN2 (fast clock state): best 48.6 us, typical 48.6-51 us, vs
the 84.8 us baseline. Engine budget: DVE GATED stream 31.55 us, stable to
+-10 ns across runs (64 ops: 19.2 us at the 2X_1PORT peak of 2 elem/lane/
cycle + ~188 ns/op custom-dispatch+SBUF-roundtrip overhead; op count is
forced — each strip has distinct per-partition consts, and v3 hardware
has no spare delay chain to page them), head ~11.5-12.2 us (~7 us NEFF
boot + load receipts), tail ~4.8 us (drain + receipt + ~2.5 us teardown
barrier). All run-to-run variance is DMA-side (load receipt latency at
the head, HBM write receipt/congestion in the drain); the device also has
a ~20% DVFS throttle state on some runs.
"""

import sys
import types

import ml_dtypes
import numpy as np

B, C, N, E = 2, 64, 1024, 32
NCORES = 8
EPC = B * E // NCORES  # 8 channels per core
P = 128
NB = N // P  # 8 row blocks
H = N // 2  # matmul moving free-dim limit is 512
NQ = NB // 2  # 4 packed strip-pairs per channel
W = 9 * P  # 1152: strip r (8-r blocks) + strip 7-r (r+1 blocks)

# Engage the DVE 2X_1PORT perf mode for the custom op (bf16 packed operands).
USE_2X = True
_CACHE = {}


def _ensure_hook_shim():
    """Make trace=True safe even when antenv.axon_hooks is absent."""
    try:
        import antenv.axon_hooks  # noqa: F401
    except ImportError:
        mod = types.ModuleType("antenv.axon_hooks")
        mod.get_axon_ntff_profile_hook = lambda: None
        mod.set_axon_ntff_profile_hook = lambda h: None
        sys.modules["antenv.axon_hooks"] = mod


def _build_2x_uop(base):
    """2X_1PORT program for out = max(in0,s0)*max(in1,s1).

    In 2X mode each port read delivers a packed pair of bf16 elements; the
    low element enters via SRC_0/SRC_1, the high one via SRC_0_HI/SRC_1_HI.
    Two copies of the max/max/mul chain are placed across the 8 ALU blocks;
    results are written packed via WR0_LO / WR0_HI. Per-partition scalar
    consts are shared by both chains (both elements are in the same row).
    """
    from concourse.dve_uop import (
        AluInp, AluOp, DelayInp, ENABLE, InpSel, OutPath, OutSel, UopConfig,
    )

    u = UopConfig()
    # lane j>=1 feeds delay chain j-1 at block 0
    u.enable_input(InpSel.SRC_0, 1)      # chain 0
    u.enable_input(InpSel.CONST_0, 2)    # chain 1
    u.enable_input(InpSel.SRC_1, 3)      # chain 2
    u.enable_input(InpSel.CONST_1, 4)    # chain 3
    u.enable_input(InpSel.SRC_0_HI, 5)   # chain 4
    u.enable_input(InpSel.SRC_1_HI, 6)   # chain 5
    u.require_inp0 = ENABLE
    u.require_inp1 = ENABLE
    u.trigger = base.trigger
    u.next_uop = (0, 0, 0)
    u.repeat_count = base.repeat_count

    dp = u.datapath_config
    # low chain
    dp[0].enable_alu(AluOp.MAX, AluInp.PREV_DELAY_0, AluInp.PREV_DELAY_1)
    dp[0].pass_through_delay(1, 2, 3, 4, 5)
    dp[1].enable_alu(AluOp.MAX, AluInp.PREV_DELAY_2, AluInp.PREV_DELAY_3)
    dp[1].enable_delay_from_src(DelayInp.PREV_ALU_OUT, 0)  # max_v_lo
    dp[1].pass_through_delay(1, 3, 4, 5)
    dp[2].enable_alu(AluOp.MULTIPLY, AluInp.PREV_DELAY_0, AluInp.PREV_ALU_OUT)
    dp[2].pass_through_delay(1, 3, 4, 5)
    # high chain (consts still on chains 1 and 3)
    dp[3].enable_alu(AluOp.MAX, AluInp.PREV_DELAY_4, AluInp.PREV_DELAY_1)
    dp[3].enable_delay_from_src(DelayInp.PREV_ALU_OUT, 0)  # out_lo
    dp[3].pass_through_delay(3, 5)
    dp[4].enable_alu(AluOp.MAX, AluInp.PREV_DELAY_5, AluInp.PREV_DELAY_3)
    dp[4].enable_delay_from_src(DelayInp.PREV_ALU_OUT, 1)  # max_v_hi
    dp[4].pass_through_delay(0)
    dp[5].enable_alu(AluOp.MULTIPLY, AluInp.PREV_DELAY_1, AluInp.PREV_ALU_OUT)
    dp[5].pass_through_delay(0)
    dp[6].pass_through_alu()
    dp[6].pass_through_delay(0)
    dp[7].pass_through_alu()
    dp[7].pass_through_delay(0)

    u.out[OutPath.WR0_LO] = OutSel.DELAY_0
    u.out_enable[OutPath.WR0_LO] = ENABLE
    u.out[OutPath.WR0_HI] = OutSel.ALU_OUT
    u.out_enable[OutPath.WR0_HI] = ENABLE
    return u


def _register_gated_maxmul():
    """Register the fused out = max(in0,s0)*max(in1,s1) custom DVE op."""
    import concourse.dve_ops as dve_ops
    from concourse.dve_ops import DveOp, OPS, has_src1, _COMPILE_CACHE
    from concourse.dve_spec import C0, C1, Spec, Src0, Src1, lower, maxx
    from concourse.dve_uop import DveOpSpec

    for op in OPS:
        if op.name == "GATED_MAXMUL_ANT":
            return op

    spec = Spec(
        body=maxx(Src0, C0) * maxx(Src1, C1),
        reference=lambda in0, in1, s0, s1, imm2: np.maximum(in0, s0)
        * np.maximum(in1, s1),
    )
    op = DveOp("GATED_MAXMUL_ANT", spec, subdim=False, uops_sha={})
    OPS.append(op)
    # Rebuild the registry views that were snapshotted at import time.
    dve_ops.CUSTOM_DVE_SPECS[op.name] = op.spec
    opcode = dve_ops._CUSTOM_DVE_ROW_BASE + len(OPS) - 1
    assert opcode < 0x20
    dve_ops._SUB_OPCODE_FOR_NAME[op.name] = opcode
    # Pre-seed the compile cache with a spec that (optionally) carries the
    # perf-mode uop programs; compile() then returns it without the sha check.
    for ver in ("v3", "v4"):
        uops = lower(spec, ver=ver)
        kw = {}
        if USE_2X:
            kw = dict(uops_2x=[_build_2x_uop(uops[0])])
        s = DveOpSpec(
            name=op.name, opcode=opcode, uops=uops,
            rd1_en=has_src1(spec), **kw,
        )
        op.uops_sha[ver] = s.sha(ver)
        _COMPILE_CACHE[(op.name, ver)] = s
    return op


def _build_program():
    import concourse.bacc as bacc
    import concourse.mybir as mybir
    import concourse.tile as tile

    f32 = mybir.dt.float32
    bf16 = mybir.dt.bfloat16
    AF = mybir.ActivationFunctionType

    gated_op = _register_gated_maxmul()

    nc = bacc.Bacc("TRN2", target_bir_lowering=False, debug=False, num_devices=NCORES)

    # e1w = emb cols 512..1024 with the [C,40] column-weight matrix
    # appended; w0 = channel 0's replicated v|g weight slices. Merged so
    # each head-path dependency is ONE dma (one trigger, one sem).
    e1w = nc.declare_dram_parameter("e1w", [C, H + 40], bf16, isOutput=False)
    emb0 = nc.declare_dram_parameter("emb0", [C, H], bf16, isOutput=False)
    w0 = nc.declare_dram_parameter("w0", [C, 2 * P], bf16, isOutput=False)
    wrepv = nc.declare_dram_parameter("wrepv", [C, N], bf16, isOutput=False)
    wrepg = nc.declare_dram_parameter("wrepg", [C, N], bf16, isOutput=False)
    th5bc = nc.declare_dram_parameter("th5bc", [P, NB * 2 * EPC], f32, isOutput=False)
    out = nc.declare_dram_parameter("out", [EPC, P, NQ, W], bf16, isOutput=True)
    diag = nc.declare_dram_parameter("diag", [P, NB * EPC], f32, isOutput=True)

    def custom(out_ap, in0, in1, s0, s1):
        bi = nc.vector._custom_dve(gated_op, out=out_ap, in0=in0, in1=in1, s0=s0, s1=s1)
        if USE_2X:
            bi.ins.perf_max = 1  # engine may escalate to 2X_1PORT
        return bi

    with tile.TileContext(nc, pool_alloc_mode="queue") as tc:
        with (
            tc.tile_pool(name="const", bufs=1) as cpool,
            tc.tile_pool(name="rows", bufs=1) as rpool,
        ):
            # Loads: trigger cost is ~0.6-0.8us each on the issuing
            # sequencer. Sync carries the replication dependencies (emb,
            # then w for the column matmuls); scalar carries the rest in
            # parallel while its ACT engine does the sigmoid table load.
            sb_warm = cpool.tile([1, EPC], f32)
            nc.vector.memset(sb_warm[:], 0.0)
            # Warm the ACT sigmoid table first, during the input loads.
            nc.scalar.activation(sb_warm[:], sb_warm[:], AF.Sigmoid)
            # h1 (cols 512..1024) first: the whole high-half dependency
            # chain (replication, columns r4-7, the narrow B-strips) can
            # run before the low half even lands. DMA-write deps are
            # tile-granular, so every independently-consumed load gets its
            # own tile, loaded by exactly one dma.
            # The two head-critical loads go SWDGE via the GpSimd queue:
            # it clears the entry barrier ~0.9us before Sync/Scalar, so
            # their data lands that much earlier. (No DVE-2X port-contention
            # risk: these complete long before the first custom op.)
            sb_w0 = cpool.tile([C, 2 * P], bf16)
            nc.gpsimd.dma_start(out=sb_w0[:], in_=w0[:])
            sb_e1w = cpool.tile([C, H + 40], bf16)
            nc.gpsimd.dma_start(out=sb_e1w[:], in_=e1w[:])
            sb_emb0 = cpool.tile([C, H], bf16)
            nc.sync.dma_start(out=sb_emb0[:], in_=emb0[:])
            sb_wrepv = cpool.tile([C, N], bf16)
            nc.scalar.dma_start(out=sb_wrepv[:, P:N], in_=wrepv[:, P:N])
            sb_wrepg = cpool.tile([C, N], bf16)
            nc.scalar.dma_start(out=sb_wrepg[:, P:N], in_=wrepg[:, P:N])
            sb_th5bc = cpool.tile([P, NB, 2 * EPC], f32)
            nc.scalar.dma_start(out=sb_th5bc[:], in_=th5bc[:])
            sb_emb1 = sb_e1w[:, 0:H]
            sb_w = sb_e1w[:, H:H + 40]

            # Column-space per-partition scalars: vgc[:, r, 0:8] holds
            # max(t'1, 0) at node r*128+p, [:, r, 8:16] max(sigmoid(t'2), .5).
            sb_vgc = rpool.tile([P, NB, 2 * EPC], f32)
            sb_gs = rpool.tile([P, NB, EPC], f32)
            sb_u = rpool.tile([P, NB, 2 * EPC], f32)
            sb_s2 = rpool.tile([P, NB, EPC], f32)
            sb_pd = rpool.tile([P, NB, EPC], f32)  # true diag d1*d2

            with (
                tc.tile_pool(name="psum", bufs=3, space="PSUM") as pp,
                tc.tile_pool(name="colps", bufs=1, space="PSUM") as cp,
                tc.tile_pool(name="jrepsb", bufs=3) as jsb,
                tc.tile_pool(name="work", bufs=3) as wp,
            ):
                ps_c = cp.tile([P, NB, 40], f32, tag="ps_c")

                def repl_half(ch, h, vj, gj):
                    """Replicated-weight matmul for one emb half: ps[p, j] =
                    t'_k[ch, j] for every partition p, straight from emb.
                    The g-side sigmoid rides the ACT PSUM->SBUF drain."""
                    cs = slice(ch * P, (ch + 1) * P)
                    hs = slice(h * H, (h + 1) * H)
                    sb_embh = sb_emb1 if h == 1 else sb_emb0[:]
                    lv = sb_w0[:, 0:P] if ch == 0 else sb_wrepv[:, cs]
                    lg = sb_w0[:, P:2 * P] if ch == 0 else sb_wrepg[:, cs]
                    ps_vh = pp.tile([P, H], f32, tag="ps_vh")
                    nc.tensor.matmul(
                        ps_vh[:], lhsT=lv,
                        rhs=sb_embh, start=True, stop=True,
                    )
                    if ch == 0 and h == 1:
                        # DVE is idle during the head; its queue runs this
                        # copy while ACT starts on the g-side sigmoid.
                        nc.vector.tensor_scalar_add(vj[:, hs], ps_vh[:], 0.0)
                    else:
                        nc.scalar.copy(vj[:, hs], ps_vh[:])
                    ps_gh = pp.tile([P, H], f32, tag="ps_gh")
                    nc.tensor.matmul(
                        ps_gh[:], lhsT=lg,
                        rhs=sb_embh, start=True, stop=True,
                    )
                    nc.scalar.activation(gj[:, hs], ps_gh[:], AF.Sigmoid)

                def replicate(ch):
                    vj = jsb.tile([P, N], bf16, tag="sb_vj")
                    gj = jsb.tile([P, N], bf16, tag="sb_gj")
                    repl_half(ch, 1, vj, gj)
                    repl_half(ch, 0, vj, gj)
                    return vj, gj

                def cols_mm(h):
                    """Columns: tcol[p, r, k] = t'_k[r*128+p] via emb-block
                    matmuls + the ACT sigmoid for the g-side scalars."""
                    rs = slice(h * 4, h * 4 + 4)
                    for r in range(h * 4, h * 4 + 4):
                        lo = (r - h * 4) * P
                        lhsT = (sb_e1w[:, lo:lo + P] if h == 1
                                else sb_emb0[:, lo:lo + P])
                        nc.tensor.matmul(
                            ps_c[:, r, :], lhsT=lhsT,
                            rhs=sb_w, start=True, stop=True,
                        )
                    nc.scalar.activation(sb_gs[:, rs, :], ps_c[:, rs, 32:40],
                                         AF.Sigmoid)

                def cols_dve(h):
                    """Per-partition scalars for rows h*512..h*512+511:
                    v-side max(t'1, 0); g-side max(sigmoid(t'2), 0.5)."""
                    rs = slice(h * 4, h * 4 + 4)
                    nc.vector.tensor_scalar_max(
                        sb_vgc[:, rs, 0:EPC], ps_c[:, rs, 0:EPC], 0.0,
                    )
                    nc.vector.tensor_scalar_max(
                        sb_vgc[:, rs, EPC:], sb_gs[:, rs, :], 0.5,
                    )

                def gated(o2, sb_vj, sb_gj, ch, q, part):
                    wq = (NB - q) * P
                    if part == 0:  # strip q, cols q*128..N
                        custom(
                            o2[:, q, 0:wq],
                            sb_vj[:, q * P:N], sb_gj[:, q * P:N],
                            sb_vgc[:, q, ch:ch + 1],
                            sb_vgc[:, q, EPC + ch:EPC + ch + 1],
                        )
                    else:  # strip 7-q, cols (7-q)*128..N
                        rq = NB - 1 - q
                        custom(
                            o2[:, q, wq:W],
                            sb_vj[:, rq * P:N], sb_gj[:, rq * P:N],
                            sb_vgc[:, rq, ch:ch + 1],
                            sb_vgc[:, rq, EPC + ch:EPC + ch + 1],
                        )

                # Channel 0 head: the full high-half chain (replication h1,
                # columns r4-7, scalars) is emitted before anything touches
                # the low half, so the narrow B-strips (cols >= 512 only)
                # start while h0 still flows through PE/ACT. The r0-3 DVE
                # scalar clips are emitted after the B-strips (in-order DVE
                # queue must not wait on the h0 chain before them).
                vj0 = jsb.tile([P, N], bf16, tag="sb_vj")
                gj0 = jsb.tile([P, N], bf16, tag="sb_gj")
                o2 = wp.tile([P, NQ, W], bf16, tag="o2")
                repl_half(0, 1, vj0, gj0)
                cols_mm(1)
                cols_dve(1)
                repl_half(0, 0, vj0, gj0)
                cols_mm(0)
                for q in range(NQ):
                    gated(o2, vj0, gj0, 0, q, 1)        # strips 7..4
                cols_dve(0)
                for q in range(NQ):
                    gated(o2, vj0, gj0, 0, q, 0)        # strips 0..3
                nc.sync.dma_start(out=out[0], in_=o2[:])

                # True-diagonal path, staged across the first two channel
                # windows so no engine's in-order queue ever waits across a
                # GATED batch: u = t' + th5 on DVE after ch0, sigmoid on
                # ACT + fused max*max on DVE after ch1.
                # Only the g-side u is needed early (ch1's ACT sigmoid reads
                # it); the v-side add joins the end-of-stream diag chain.
                ALU = mybir.AluOpType
                nc.vector.scalar_tensor_tensor(
                    sb_u[:, :, EPC:], ps_c[:, :, 32:40], 1.0,
                    sb_th5bc[:, :, EPC:], op0=ALU.mult, op1=ALU.add,
                )

                for ch in range(1, EPC - 2):
                    sb_vj, sb_gj = replicate(ch)

                    # Upper-triangle strips, packed in constant-width pairs:
                    # o2[:, q, 0:wq]  = strip q   (rows q*128+p, cols q*128..N)
                    # o2[:, q, wq:W]  = strip 7-q (rows (7-q)*128+p, cols (7-q)*128..N)
                    o2 = wp.tile([P, NQ, W], bf16, tag="o2")
                    for q in range(NQ):
                        gated(o2, sb_vj, sb_gj, ch, q, 0)
                        gated(o2, sb_vj, sb_gj, ch, q, 1)
                    # One 1.125 MiB store per channel on the otherwise-
                    # idle Sync queue; all 16 SDMA engines participate.
                    nc.sync.dma_start(out=out[ch], in_=o2[:])

                    if ch == 1:
                        # Diagonal g-side sigmoid rides ACT's slack here;
                        # the fused max*max and the diag store are emitted
                        # after the last GATED so they hide under the tail
                        # DMA drain instead of sitting in the DVE stream.
                        nc.scalar.activation(sb_s2[:], sb_u[:, :, EPC:],
                                             AF.Sigmoid)

                # Final two channels interleaved at PAIR granularity with
                # contiguous 288 KiB pair stores: ch6's bytes ship inside
                # its own compute window and ch7's spread over ~2 windows,
                # so the post-compute DMA drain is one pair, not 2.25 MiB.
                # (Strip-granular stores were tried and are slower: the
                # odd non-contiguous shapes cut SDMA efficiency.)
                c6, c7 = EPC - 2, EPC - 1
                vj6, gj6 = replicate(c6)
                vj7, gj7 = replicate(c7)
                o6 = wp.tile([P, NQ, W], bf16, tag="o2")
                o7 = wp.tile([P, NQ, W], bf16, tag="o2")
                nst = 0

                def pair_store(ch, o2, q):
                    nonlocal nst
                    eng = nc.sync if nst % 2 == 0 else nc.scalar
                    nst += 1
                    eng.dma_start(out=out[ch, :, q:q + 1, :],
                                  in_=o2[:, q:q + 1, :])

                # ch6 pairs 0,1 first (repl7's ACT copies finish ~1.5us into
                # this window), then alternate, ch7's last pair last.
                order = [(c6, o6, vj6, gj6, 0), (c6, o6, vj6, gj6, 1),
                         (c7, o7, vj7, gj7, 0), (c6, o6, vj6, gj6, 2),
                         (c7, o7, vj7, gj7, 1), (c6, o6, vj6, gj6, 3),
                         (c7, o7, vj7, gj7, 2), (c7, o7, vj7, gj7, 3)]
                for ch, o2, vj, gj, q in order:
                    gated(o2, vj, gj, ch, q, 0)
                    gated(o2, vj, gj, ch, q, 1)
                    pair_store(ch, o2, q)

                # Diagonal d1*d2 = max(u1,0)*max(sigmoid(u2),0.5): the u1
                # add and one custom-DVE op with immediate scalars, run
                # concurrently with the final pair's DMA drain; the 32 KiB
                # diag store's receipt lands with the last pair store's.
                nc.vector.scalar_tensor_tensor(
                    sb_u[:, :, 0:EPC], ps_c[:, :, 0:EPC], 1.0,
                    sb_th5bc[:, :, 0:EPC], op0=ALU.mult, op1=ALU.add,
                )
                custom(sb_pd[:], sb_u[:, :, 0:EPC], sb_s2[:], 0.0, 0.5)
                nc.sync.dma_start(out=diag[:], in_=sb_pd[:])

    nc.compile()
    return nc


def _get_program():
    if "nc" not in _CACHE:
        _CACHE["nc"] = _build_program()
    return _CACHE["nc"]


def kernel(**inputs):
    _ensure_hook_shim()
    from concourse.bass_utils import run_bass_kernel_spmd

    bf = ml_dtypes.bfloat16
    emb = np.ascontiguousarray(np.asarray(inputs["emb"], dtype=np.float32)).astype(bf)
    th12_1 = np.asarray(inputs["th12_1"], dtype=np.float32)
    th12_2 = np.asarray(inputs["th12_2"], dtype=np.float32)
    th5_1 = np.asarray(inputs["th5_1"], dtype=np.float32)
    th5_2 = np.asarray(inputs["th5_2"], dtype=np.float32)

    in_maps = []
    for k in range(NCORES):
        b = k // (NCORES // B)
        e0 = (k % (NCORES // B)) * EPC
        # The 2x of the reference's "m + m" is folded into the weights.
        w2_1 = (2.0 * th12_1[e0:e0 + EPC]).astype(bf)  # [EPC, C]
        w2_2 = (2.0 * th12_2[e0:e0 + EPC]).astype(bf)
        wm = np.zeros((C, 40), dtype=bf)
        wm[:, 0:EPC] = w2_1.T
        wm[:, 32:40] = w2_2.T
        # wrep[c, ch*128+m] = 2*th12[e0+ch, c] for all m (replicated cols).
        wrepv = np.repeat(w2_1.T[:, :, None], P, axis=2).reshape(C, N)
        wrepg = np.repeat(w2_2.T[:, :, None], P, axis=2).reshape(C, N)
        th5cat = np.concatenate([th5_1[e0:e0 + EPC], th5_2[e0:e0 + EPC]])  # [16]
        th5bc = np.tile(th5cat[None, :], (P, NB)).astype(np.float32)  # [128, 128]
        in_maps.append(
            {
                "e1w": np.ascontiguousarray(
                    np.concatenate([emb[b][:, H:N], wm], axis=1)),
                "emb0": np.ascontiguousarray(emb[b][:, 0:H]),
                "w0": np.ascontiguousarray(
                    np.concatenate([wrepv[:, 0:P], wrepg[:, 0:P]], axis=1)),
                "wrepv": np.ascontiguousarray(wrepv),
                "wrepg": np.ascontiguousarray(wrepg),
                "th5bc": th5bc,
            }
        )

    nc = _get_program()
    res = run_bass_kernel_spmd(nc, in_maps, core_ids=list(range(NCORES)))
    _CACHE["last_result"] = res

    out = np.empty((B, E, N, N), dtype=np.float32)
    for k in range(NCORES):
        b = k // (NCORES // B)
        e0 = (k % (NCORES // B)) * EPC
        a = np.asarray(res.results[k]["out"], dtype=np.float32)  # [EPC,P,NQ,W]
        dg = np.asarray(res.results[k]["diag"], dtype=np.float32)  # [P, NB*EPC]
        oc = out[b, e0:e0 + EPC]  # [EPC, N, N] view
        # Unpack the paired upper-triangle strips.
        for q in range(NQ):
            wq = (NB - q) * P
            rq = NB - 1 - q
            oc[:, q * P:(q + 1) * P, q * P:] = a[:, :, q, 0:wq]
            oc[:, rq * P:(rq + 1) * P, rq * P:] = a[:, :, q, wq:W]
        # Mirror the lower triangle (the matrix is symmetric).
        o6 = oc.reshape(EPC, NB, P, NB, P)
        for r in range(NB):
            for c in range(r):
                o6[:, r, :, c, :] = o6[:, c, :, r, :].transpose(0, 2, 1)
        # Place the true diagonal (f32, exact).
        dv = dg.reshape(P, NB, EPC).transpose(2, 1, 0).reshape(EPC, N)
        oc.reshape(EPC, N * N)[:, ::N + 1] = dv
    return out


# revision 56
# speedup vs baseline: 1.0109x; 1.0109x over previous
"""Trainium2 Bass/Tile kernel for the GatedNode2Edge op.

Computes, for emb (B,C,N), th12_* (E,C), th5_* (E,):
    t_k  = th12_k @ emb[b]                      (E,N)
    m_k  = max(t_k[:,i], t_k[:,j]) pairwise     (E,N,N)
    adj  = relu(2*m_1 + th5_1*I)
    gate = sigmoid(relu(2*m_2 + th5_2*I))
    out  = adj * gate                           (B,E,N,N)

Sharding: the 64 (b,e) channels are split 8-per-core across 8 NeuronCores.

Math restructuring (off-diagonal): with the 2x folded into the weights
(t' = 2t), and using that max() commutes with the monotone sigmoid and
relu(x) = max(x, 0), sigmoid(relu(x)) = max(sigmoid(x), 0.5):
    adj[i,j]  = max(t'1_j,          max(t'1_i, 0))
    gate[i,j] = max(sigmoid(t'2_j), max(sigmoid(t'2_i), 0.5))
so the REPLICATED (column-varying) operands need no clipping at all — the
clips live entirely in the per-partition scalars. One fused custom-DVE op
per output strip:
    out = maxx(Src0, C0) * maxx(Src1, C1)
Src0/Src1 = t'1 / sigmoid(t'2) rows replicated across partitions, built
by a replicated-weight matmul on the PE (lhsT[c, m] = 2*th12_k[ch, c] for
all m) directly from emb — no row-space phase at all. The sigmoid is
applied by ACT during the PSUM->SBUF drain (free). C0/C1 are per-partition
column slices. The custom op carries a hand-authored 2X_1PORT uop program
(two parallel max/max/mul chains over the packed bf16 pair via SRC_*_HI,
written through WR0_LO/WR0_HI).

The output matrix is SYMMETRIC (max is), so the device only computes the
36 upper-triangle [128,128] blocks of each channel's (N,N) matrix. Row
strip r (rows r*128..r*128+127, cols r*128..N) is paired with strip 7-r
so each pair packs to a constant 9*128=1152 columns: the o2 tile is
[128, 4, 1152] = 9 KiB/partition, stored with ONE dma_start per channel
(1.125 MiB, all 16 SDMA engines) on the otherwise-idle Sync queue —
dma_start triggers cost ~0.6-0.8us of issuing-sequencer time each, which
is why the stores are batched this coarsely. The host mirrors the lower
triangle during unsharding (pure data placement, like the bf16->f32 cast).

The true diagonal d1*d2 (d1 = relu(t'1+th5_1), d2 = sigmoid(relu(t'2+th5_2))
= max(sigmoid(t'2+th5_2), 0.5)) is computed on-device as one extra custom-op
call on a [128, 8, 8] f32 tile and exported as a second tiny output; the
host scatters it onto the diagonals.

Startup is load-receipt-bound: the emb high half rides one merged dma with
the column-weight matrix, channel 0's replication weights ride a dedicated
2.5 KiB load (DMA-write deps are tile-granular — one dma per head tile),
and channel 0 runs its narrow high-half B-strips while the low half is
still in flight (its v-copy runs on the then-idle DVE). The tail is
DMA-drain-bound: the final two channels are interleaved at strip-pair
granularity with contiguous 288 KiB pair stores on alternating queues, so
their combined 2.25 MiB spreads over both compute windows and the final
drain is one pair. (Finer strip-granular stores were tried and are slower:
odd non-contiguous shapes cut SDMA efficiency; so is a quarter-granularity
channel-0 head: it shortens the head but lengthens the DVE stream more.)

Everything on-device is bf16 (harness tolerance 2e-2 >> bf16 rounding); the
host converts back to f32. This halves HBM write traffic and halves matmul
LDWEIGHTS time.

Measured on TRN2 (fast clock state): best 48.6 us, typical 48.6-51 us, vs
the 84.8 us baseline. Engine budget: DVE GATED stream 31.55 us, stable to
+-10 ns across runs (64 ops: 19.2 us at the 2X_1PORT peak of 2 elem/lane/
cycle + ~188 ns/op custom-dispatch+SBUF-roundtrip overhead; op count is
forced — each strip has distinct per-partition consts, and v3 hardware
has no spare delay chain to page them), head ~11.5-12.2 us (~7 us NEFF
boot + load receipts), tail ~4.8 us (drain + receipt + ~2.5 us teardown
barrier). All run-to-run variance is DMA-side (load receipt latency at
the head, HBM write receipt/congestion in the drain); the device also has
a ~20% DVFS throttle state on some runs.
"""

import sys
import types

import ml_dtypes
import numpy as np

B, C, N, E = 2, 64, 1024, 32
NCORES = 8
EPC = B * E // NCORES  # 8 channels per core
P = 128
NB = N // P  # 8 row blocks
H = N // 2  # matmul moving free-dim limit is 512
NQ = NB // 2  # 4 packed strip-pairs per channel
W = 9 * P  # 1152: strip r (8-r blocks) + strip 7-r (r+1 blocks)

# Engage the DVE 2X_1PORT perf mode for the custom op (bf16 packed operands).
USE_2X = True
_CACHE = {}


def _ensure_hook_shim():
    """Make trace=True safe even when antenv.axon_hooks is absent."""
    try:
        import antenv.axon_hooks  # noqa: F401
    except ImportError:
        mod = types.ModuleType("antenv.axon_hooks")
        mod.get_axon_ntff_profile_hook = lambda: None
        mod.set_axon_ntff_profile_hook = lambda h: None
        sys.modules["antenv.axon_hooks"] = mod


def _build_2x_uop(base):
    """2X_1PORT program for out = max(in0,s0)*max(in1,s1).

    In 2X mode each port read delivers a packed pair of bf16 elements; the
    low element enters via SRC_0/SRC_1, the high one via SRC_0_HI/SRC_1_HI.
    Two copies of the max/max/mul chain are placed across the 8 ALU blocks;
    results are written packed via WR0_LO / WR0_HI. Per-partition scalar
    consts are shared by both chains (both elements are in the same row).
    """
    from concourse.dve_uop import (
        AluInp, AluOp, DelayInp, ENABLE, InpSel, OutPath, OutSel, UopConfig,
    )

    u = UopConfig()
    # lane j>=1 feeds delay chain j-1 at block 0
    u.enable_input(InpSel.SRC_0, 1)      # chain 0
    u.enable_input(InpSel.CONST_0, 2)    # chain 1
    u.enable_input(InpSel.SRC_1, 3)      # chain 2
    u.enable_input(InpSel.CONST_1, 4)    # chain 3
    u.enable_input(InpSel.SRC_0_HI, 5)   # chain 4
    u.enable_input(InpSel.SRC_1_HI, 6)   # chain 5
    u.require_inp0 = ENABLE
    u.require_inp1 = ENABLE
    u.trigger = base.trigger
    u.next_uop = (0, 0, 0)
    u.repeat_count = base.repeat_count

    dp = u.datapath_config
    # low chain
    dp[0].enable_alu(AluOp.MAX, AluInp.PREV_DELAY_0, AluInp.PREV_DELAY_1)
    dp[0].pass_through_delay(1, 2, 3, 4, 5)
    dp[1].enable_alu(AluOp.MAX, AluInp.PREV_DELAY_2, AluInp.PREV_DELAY_3)
    dp[1].enable_delay_from_src(DelayInp.PREV_ALU_OUT, 0)  # max_v_lo
    dp[1].pass_through_delay(1, 3, 4, 5)
    dp[2].enable_alu(AluOp.MULTIPLY, AluInp.PREV_DELAY_0, AluInp.PREV_ALU_OUT)
    dp[2].pass_through_delay(1, 3, 4, 5)
    # high chain (consts still on chains 1 and 3)
    dp[3].enable_alu(AluOp.MAX, AluInp.PREV_DELAY_4, AluInp.PREV_DELAY_1)
    dp[3].enable_delay_from_src(DelayInp.PREV_ALU_OUT, 0)  # out_lo
    dp[3].pass_through_delay(3, 5)
    dp[4].enable_alu(AluOp.MAX, AluInp.PREV_DELAY_5, AluInp.PREV_DELAY_3)
    dp[4].enable_delay_from_src(DelayInp.PREV_ALU_OUT, 1)  # max_v_hi
    dp[4].pass_through_delay(0)
    dp[5].enable_alu(AluOp.MULTIPLY, AluInp.PREV_DELAY_1, AluInp.PREV_ALU_OUT)
    dp[5].pass_through_delay(0)
    dp[6].pass_through_alu()
    dp[6].pass_through_delay(0)
    dp[7].pass_through_alu()
    dp[7].pass_through_delay(0)

    u.out[OutPath.WR0_LO] = OutSel.DELAY_0
    u.out_enable[OutPath.WR0_LO] = ENABLE
    u.out[OutPath.WR0_HI] = OutSel.ALU_OUT
    u.out_enable[OutPath.WR0_HI] = ENABLE
    return u


def _register_gated_maxmul():
    """Register the fused out = max(in0,s0)*max(in1,s1) custom DVE op."""
    import concourse.dve_ops as dve_ops
    from concourse.dve_ops import DveOp, OPS, has_src1, _COMPILE_CACHE
    from concourse.dve_spec import C0, C1, Spec, Src0, Src1, lower, maxx
    from concourse.dve_uop import DveOpSpec

    for op in OPS:
        if op.name == "GATED_MAXMUL_ANT":
            return op

    spec = Spec(
        body=maxx(Src0, C0) * maxx(Src1, C1),
        reference=lambda in0, in1, s0, s1, imm2: np.maximum(in0, s0)
        * np.maximum(in1, s1),
    )
    op = DveOp("GATED_MAXMUL_ANT", spec, subdim=False, uops_sha={})
    OPS.append(op)
    # Rebuild the registry views that were snapshotted at import time.
    dve_ops.CUSTOM_DVE_SPECS[op.name] = op.spec
    opcode = dve_ops._CUSTOM_DVE_ROW_BASE + len(OPS) - 1
    assert opcode < 0x20
    dve_ops._SUB_OPCODE_FOR_NAME[op.name] = opcode
    # Pre-seed the compile cache with a spec that (optionally) carries the
    # perf-mode uop programs; compile() then returns it without the sha check.
    for ver in ("v3", "v4"):
        uops = lower(spec, ver=ver)
        kw = {}
        if USE_2X:
            kw = dict(uops_2x=[_build_2x_uop(uops[0])])
        s = DveOpSpec(
            name=op.name, opcode=opcode, uops=uops,
            rd1_en=has_src1(spec), **kw,
        )
        op.uops_sha[ver] = s.sha(ver)
        _COMPILE_CACHE[(op.name, ver)] = s
    return op


def _build_program():
    import concourse.bacc as bacc
    import concourse.mybir as mybir
    import concourse.tile as tile

    f32 = mybir.dt.float32
    bf16 = mybir.dt.bfloat16
    AF = mybir.ActivationFunctionType

    gated_op = _register_gated_maxmul()

    nc = bacc.Bacc("TRN2", target_bir_lowering=False, debug=False, num_devices=NCORES)

    # e1w = emb cols 512..1024 with the [C,40] column-weight matrix
    # appended; w0 = channel 0's replicated v|g weight slices. Merged so
    # each head-path dependency is ONE dma (one trigger, one sem).
    e1w = nc.declare_dram_parameter("e1w", [C, H + 40], bf16, isOutput=False)
    emb0 = nc.declare_dram_parameter("emb0", [C, H], bf16, isOutput=False)
    w0 = nc.declare_dram_parameter("w0", [C, 2 * P], bf16, isOutput=False)
    wrepv = nc.declare_dram_parameter("wrepv", [C, N], bf16, isOutput=False)
    wrepg = nc.declare_dram_parameter("wrepg", [C, N], bf16, isOutput=False)
    th5bc = nc.declare_dram_parameter("th5bc", [P, NB * 2 * EPC], f32, isOutput=False)
    out = nc.declare_dram_parameter("out", [EPC, P, NQ, W], bf16, isOutput=True)
    diag = nc.declare_dram_parameter("diag", [P, NB * EPC], f32, isOutput=True)

    def custom(out_ap, in0, in1, s0, s1):
        bi = nc.vector._custom_dve(gated_op, out=out_ap, in0=in0, in1=in1, s0=s0, s1=s1)
        if USE_2X:
            bi.ins.perf_max = 1  # engine may escalate to 2X_1PORT
        return bi

    with tile.TileContext(nc, pool_alloc_mode="queue") as tc:
        with (
            tc.tile_pool(name="const", bufs=1) as cpool,
            tc.tile_pool(name="rows", bufs=1) as rpool,
        ):
            # Loads: trigger cost is ~0.6-0.8us each on the issuing
            # sequencer. Sync carries the replication dependencies (emb,
            # then w for the column matmuls); scalar carries the rest in
            # parallel while its ACT engine does the sigmoid table load.
            sb_warm = cpool.tile([1, EPC], f32)
            nc.vector.memset(sb_warm[:], 0.0)
            # Warm the ACT sigmoid table first, during the input loads.
            nc.scalar.activation(sb_warm[:], sb_warm[:], AF.Sigmoid)
            # h1 (cols 512..1024) first: the whole high-half dependency
            # chain (replication, columns r4-7, the narrow B-strips) can
            # run before the low half even lands. DMA-write deps are
            # tile-granular, so every independently-consumed load gets its
            # own tile, loaded by exactly one dma.
            # Head-critical loads on HWDGE (sync/scalar). SWDGE via the
            # GpSimd queue was tried — it clears the entry barrier ~0.9us
            # earlier, but the Q7 descriptor-generation overhead costs more
            # than that and the head LOST ~1.6us.
            sb_e1w = cpool.tile([C, H + 40], bf16)
            nc.sync.dma_start(out=sb_e1w[:], in_=e1w[:])
            sb_emb0 = cpool.tile([C, H], bf16)
            nc.sync.dma_start(out=sb_emb0[:], in_=emb0[:])
            sb_w0 = cpool.tile([C, 2 * P], bf16)
            nc.scalar.dma_start(out=sb_w0[:], in_=w0[:])
            sb_wrepv = cpool.tile([C, N], bf16)
            nc.scalar.dma_start(out=sb_wrepv[:, P:N], in_=wrepv[:, P:N])
            sb_wrepg = cpool.tile([C, N], bf16)
            nc.scalar.dma_start(out=sb_wrepg[:, P:N], in_=wrepg[:, P:N])
            sb_th5bc = cpool.tile([P, NB, 2 * EPC], f32)
            nc.scalar.dma_start(out=sb_th5bc[:], in_=th5bc[:])
            sb_emb1 = sb_e1w[:, 0:H]
            sb_w = sb_e1w[:, H:H + 40]

            # Column-space per-partition scalars: vgc[:, r, 0:8] holds
            # max(t'1, 0) at node r*128+p, [:, r, 8:16] max(sigmoid(t'2), .5).
            sb_vgc = rpool.tile([P, NB, 2 * EPC], f32)
            sb_gs = rpool.tile([P, NB, EPC], f32)
            sb_u = rpool.tile([P, NB, 2 * EPC], f32)
            sb_s2 = rpool.tile([P, NB, EPC], f32)
            sb_pd = rpool.tile([P, NB, EPC], f32)  # true diag d1*d2

            with (
                tc.tile_pool(name="psum", bufs=3, space="PSUM") as pp,
                tc.tile_pool(name="colps", bufs=1, space="PSUM") as cp,
                tc.tile_pool(name="jrepsb", bufs=3) as jsb,
                tc.tile_pool(name="work", bufs=3) as wp,
            ):
                ps_c = cp.tile([P, NB, 40], f32, tag="ps_c")

                def repl_half(ch, h, vj, gj):
                    """Replicated-weight matmul for one emb half: ps[p, j] =
                    t'_k[ch, j] for every partition p, straight from emb.
                    The g-side sigmoid rides the ACT PSUM->SBUF drain."""
                    cs = slice(ch * P, (ch + 1) * P)
                    hs = slice(h * H, (h + 1) * H)
                    sb_embh = sb_emb1 if h == 1 else sb_emb0[:]
                    lv = sb_w0[:, 0:P] if ch == 0 else sb_wrepv[:, cs]
                    lg = sb_w0[:, P:2 * P] if ch == 0 else sb_wrepg[:, cs]
                    ps_vh = pp.tile([P, H], f32, tag="ps_vh")
                    nc.tensor.matmul(
                        ps_vh[:], lhsT=lv,
                        rhs=sb_embh, start=True, stop=True,
                    )
                    if ch == 0 and h == 1:
                        # DVE is idle during the head; its queue runs this
                        # copy while ACT starts on the g-side sigmoid.
                        nc.vector.tensor_scalar_add(vj[:, hs], ps_vh[:], 0.0)
                    else:
                        nc.scalar.copy(vj[:, hs], ps_vh[:])
                    ps_gh = pp.tile([P, H], f32, tag="ps_gh")
                    nc.tensor.matmul(
                        ps_gh[:], lhsT=lg,
                        rhs=sb_embh, start=True, stop=True,
                    )
                    nc.scalar.activation(gj[:, hs], ps_gh[:], AF.Sigmoid)

                def replicate(ch):
                    vj = jsb.tile([P, N], bf16, tag="sb_vj")
                    gj = jsb.tile([P, N], bf16, tag="sb_gj")
                    repl_half(ch, 1, vj, gj)
                    repl_half(ch, 0, vj, gj)
                    return vj, gj

                def cols_mm(h):
                    """Columns: tcol[p, r, k] = t'_k[r*128+p] via emb-block
                    matmuls + the ACT sigmoid for the g-side scalars."""
                    rs = slice(h * 4, h * 4 + 4)
                    for r in range(h * 4, h * 4 + 4):
                        lo = (r - h * 4) * P
                        lhsT = (sb_e1w[:, lo:lo + P] if h == 1
                                else sb_emb0[:, lo:lo + P])
                        nc.tensor.matmul(
                            ps_c[:, r, :], lhsT=lhsT,
                            rhs=sb_w, start=True, stop=True,
                        )
                    nc.scalar.activation(sb_gs[:, rs, :], ps_c[:, rs, 32:40],
                                         AF.Sigmoid)

                def cols_dve(h):
                    """Per-partition scalars for rows h*512..h*512+511:
                    v-side max(t'1, 0); g-side max(sigmoid(t'2), 0.5)."""
                    rs = slice(h * 4, h * 4 + 4)
                    nc.vector.tensor_scalar_max(
                        sb_vgc[:, rs, 0:EPC], ps_c[:, rs, 0:EPC], 0.0,
                    )
                    nc.vector.tensor_scalar_max(
                        sb_vgc[:, rs, EPC:], sb_gs[:, rs, :], 0.5,
                    )

                def gated(o2, sb_vj, sb_gj, ch, q, part):
                    wq = (NB - q) * P
                    if part == 0:  # strip q, cols q*128..N
                        custom(
                            o2[:, q, 0:wq],
                            sb_vj[:, q * P:N], sb_gj[:, q * P:N],
                            sb_vgc[:, q, ch:ch + 1],
                            sb_vgc[:, q, EPC + ch:EPC + ch + 1],
                        )
                    else:  # strip 7-q, cols (7-q)*128..N
                        rq = NB - 1 - q
                        custom(
                            o2[:, q, wq:W],
                            sb_vj[:, rq * P:N], sb_gj[:, rq * P:N],
                            sb_vgc[:, rq, ch:ch + 1],
                            sb_vgc[:, rq, EPC + ch:EPC + ch + 1],
                        )

                # Channel 0 head: the full high-half chain (replication h1,
                # columns r4-7, scalars) is emitted before anything touches
                # the low half, so the narrow B-strips (cols >= 512 only)
                # start while h0 still flows through PE/ACT. The r0-3 DVE
                # scalar clips are emitted after the B-strips (in-order DVE
                # queue must not wait on the h0 chain before them).
                vj0 = jsb.tile([P, N], bf16, tag="sb_vj")
                gj0 = jsb.tile([P, N], bf16, tag="sb_gj")
                o2 = wp.tile([P, NQ, W], bf16, tag="o2")
                repl_half(0, 1, vj0, gj0)
                cols_mm(1)
                cols_dve(1)
                repl_half(0, 0, vj0, gj0)
                cols_mm(0)
                for q in range(NQ):
                    gated(o2, vj0, gj0, 0, q, 1)        # strips 7..4
                cols_dve(0)
                for q in range(NQ):
                    gated(o2, vj0, gj0, 0, q, 0)        # strips 0..3
                nc.sync.dma_start(out=out[0], in_=o2[:])

                # True-diagonal path, staged across the first two channel
                # windows so no engine's in-order queue ever waits across a
                # GATED batch: u = t' + th5 on DVE after ch0, sigmoid on
                # ACT + fused max*max on DVE after ch1.
                # Only the g-side u is needed early (ch1's ACT sigmoid reads
                # it); the v-side add joins the end-of-stream diag chain.
                ALU = mybir.AluOpType
                nc.vector.scalar_tensor_tensor(
                    sb_u[:, :, EPC:], ps_c[:, :, 32:40], 1.0,
                    sb_th5bc[:, :, EPC:], op0=ALU.mult, op1=ALU.add,
                )

                for ch in range(1, EPC - 2):
                    sb_vj, sb_gj = replicate(ch)

                    # Upper-triangle strips, packed in constant-width pairs:
                    # o2[:, q, 0:wq]  = strip q   (rows q*128+p, cols q*128..N)
                    # o2[:, q, wq:W]  = strip 7-q (rows (7-q)*128+p, cols (7-q)*128..N)
                    o2 = wp.tile([P, NQ, W], bf16, tag="o2")
                    for q in range(NQ):
                        gated(o2, sb_vj, sb_gj, ch, q, 0)
                        gated(o2, sb_vj, sb_gj, ch, q, 1)
                    # One 1.125 MiB store per channel on the otherwise-
                    # idle Sync queue; all 16 SDMA engines participate.
                    nc.sync.dma_start(out=out[ch], in_=o2[:])

                    if ch == 1:
                        # Diagonal g-side sigmoid rides ACT's slack here;
                        # the fused max*max and the diag store are emitted
                        # after the last GATED so they hide under the tail
                        # DMA drain instead of sitting in the DVE stream.
                        nc.scalar.activation(sb_s2[:], sb_u[:, :, EPC:],
                                             AF.Sigmoid)

                # Final two channels interleaved at PAIR granularity with
                # contiguous 288 KiB pair stores: ch6's bytes ship inside
                # its own compute window and ch7's spread over ~2 windows,
                # so the post-compute DMA drain is one pair, not 2.25 MiB.
                # (Strip-granular stores were tried and are slower: the
                # odd non-contiguous shapes cut SDMA efficiency.)
                c6, c7 = EPC - 2, EPC - 1
                vj6, gj6 = replicate(c6)
                vj7, gj7 = replicate(c7)
                o6 = wp.tile([P, NQ, W], bf16, tag="o2")
                o7 = wp.tile([P, NQ, W], bf16, tag="o2")
                nst = 0

                def pair_store(ch, o2, q):
                    nonlocal nst
                    eng = nc.sync if nst % 2 == 0 else nc.scalar
                    nst += 1
                    eng.dma_start(out=out[ch, :, q:q + 1, :],
                                  in_=o2[:, q:q + 1, :])

                # ch6 pairs 0,1 first (repl7's ACT copies finish ~1.5us into
                # this window), then alternate, ch7's last pair last.
                order = [(c6, o6, vj6, gj6, 0), (c6, o6, vj6, gj6, 1),
                         (c7, o7, vj7, gj7, 0), (c6, o6, vj6, gj6, 2),
                         (c7, o7, vj7, gj7, 1), (c6, o6, vj6, gj6, 3),
                         (c7, o7, vj7, gj7, 2), (c7, o7, vj7, gj7, 3)]
                for ch, o2, vj, gj, q in order:
                    gated(o2, vj, gj, ch, q, 0)
                    gated(o2, vj, gj, ch, q, 1)
                    pair_store(ch, o2, q)

                # Diagonal d1*d2 = max(u1,0)*max(sigmoid(u2),0.5): the u1
                # add and one custom-DVE op with immediate scalars, run
                # concurrently with the final pair's DMA drain; the 32 KiB
                # diag store's receipt lands with the last pair store's.
                nc.vector.scalar_tensor_tensor(
                    sb_u[:, :, 0:EPC], ps_c[:, :, 0:EPC], 1.0,
                    sb_th5bc[:, :, 0:EPC], op0=ALU.mult, op1=ALU.add,
                )
                custom(sb_pd[:], sb_u[:, :, 0:EPC], sb_s2[:], 0.0, 0.5)
                nc.sync.dma_start(out=diag[:], in_=sb_pd[:])

    nc.compile()
    return nc


def _get_program():
    if "nc" not in _CACHE:
        _CACHE["nc"] = _build_program()
    return _CACHE["nc"]


def kernel(**inputs):
    _ensure_hook_shim()
    from concourse.bass_utils import run_bass_kernel_spmd

    bf = ml_dtypes.bfloat16
    emb = np.ascontiguousarray(np.asarray(inputs["emb"], dtype=np.float32)).astype(bf)
    th12_1 = np.asarray(inputs["th12_1"], dtype=np.float32)
    th12_2 = np.asarray(inputs["th12_2"], dtype=np.float32)
    th5_1 = np.asarray(inputs["th5_1"], dtype=np.float32)
    th5_2 = np.asarray(inputs["th5_2"], dtype=np.float32)

    in_maps = []
    for k in range(NCORES):
        b = k // (NCORES // B)
        e0 = (k % (NCORES // B)) * EPC
        # The 2x of the reference's "m + m" is folded into the weights.
        w2_1 = (2.0 * th12_1[e0:e0 + EPC]).astype(bf)  # [EPC, C]
        w2_2 = (2.0 * th12_2[e0:e0 + EPC]).astype(bf)
        wm = np.zeros((C, 40), dtype=bf)
        wm[:, 0:EPC] = w2_1.T
        wm[:, 32:40] = w2_2.T
        # wrep[c, ch*128+m] = 2*th12[e0+ch, c] for all m (replicated cols).
        wrepv = np.repeat(w2_1.T[:, :, None], P, axis=2).reshape(C, N)
        wrepg = np.repeat(w2_2.T[:, :, None], P, axis=2).reshape(C, N)
        th5cat = np.concatenate([th5_1[e0:e0 + EPC], th5_2[e0:e0 + EPC]])  # [16]
        th5bc = np.tile(th5cat[None, :], (P, NB)).astype(np.float32)  # [128, 128]
        in_maps.append(
            {
                "e1w": np.ascontiguousarray(
                    np.concatenate([emb[b][:, H:N], wm], axis=1)),
                "emb0": np.ascontiguousarray(emb[b][:, 0:H]),
                "w0": np.ascontiguousarray(
                    np.concatenate([wrepv[:, 0:P], wrepg[:, 0:P]], axis=1)),
                "wrepv": np.ascontiguousarray(wrepv),
                "wrepg": np.ascontiguousarray(wrepg),
                "th5bc": th5bc,
            }
        )

    nc = _get_program()
    res = run_bass_kernel_spmd(nc, in_maps, core_ids=list(range(NCORES)))
    _CACHE["last_result"] = res

    out = np.empty((B, E, N, N), dtype=np.float32)
    for k in range(NCORES):
        b = k // (NCORES // B)
        e0 = (k % (NCORES // B)) * EPC
        a = np.asarray(res.results[k]["out"], dtype=np.float32)  # [EPC,P,NQ,W]
        dg = np.asarray(res.results[k]["diag"], dtype=np.float32)  # [P, NB*EPC]
        oc = out[b, e0:e0 + EPC]  # [EPC, N, N] view
        # Unpack the paired upper-triangle strips.
        for q in range(NQ):
            wq = (NB - q) * P
            rq = NB - 1 - q
            oc[:, q * P:(q + 1) * P, q * P:] = a[:, :, q, 0:wq]
            oc[:, rq * P:(rq + 1) * P, rq * P:] = a[:, :, q, wq:W]
        # Mirror the lower triangle (the matrix is symmetric).
        o6 = oc.reshape(EPC, NB, P, NB, P)
        for r in range(NB):
            for c in range(r):
                o6[:, r, :, c, :] = o6[:, c, :, r, :].transpose(0, 2, 1)
        # Place the true diagonal (f32, exact).
        dv = dg.reshape(P, NB, EPC).transpose(2, 1, 0).reshape(EPC, N)
        oc.reshape(EPC, N * N)[:, ::N + 1] = dv
    return out


# revision 62
# speedup vs baseline: 1.0418x; 1.0306x over previous
"""Trainium2 Bass/Tile kernel for the GatedNode2Edge op.

Computes, for emb (B,C,N), th12_* (E,C), th5_* (E,):
    t_k  = th12_k @ emb[b]                      (E,N)
    m_k  = max(t_k[:,i], t_k[:,j]) pairwise     (E,N,N)
    adj  = relu(2*m_1 + th5_1*I)
    gate = sigmoid(relu(2*m_2 + th5_2*I))
    out  = adj * gate                           (B,E,N,N)

Sharding: the 64 (b,e) channels are split 8-per-core across 8 NeuronCores.

Math restructuring (off-diagonal): with the 2x folded into the weights
(t' = 2t), and using that max() commutes with the monotone sigmoid and
relu(x) = max(x, 0), sigmoid(relu(x)) = max(sigmoid(x), 0.5):
    adj[i,j]  = max(t'1_j,          max(t'1_i, 0))
    gate[i,j] = max(sigmoid(t'2_j), max(sigmoid(t'2_i), 0.5))
so the REPLICATED (column-varying) operands need no clipping at all — the
clips live entirely in the per-partition scalars. One fused custom-DVE op
per output strip:
    out = maxx(Src0, C0) * maxx(Src1, C1)
Src0/Src1 = t'1 / sigmoid(t'2) rows replicated across partitions, built
by a replicated-weight matmul on the PE (lhsT[c, m] = 2*th12_k[ch, c] for
all m) directly from emb — no row-space phase at all. The sigmoid is
applied by ACT during the PSUM->SBUF drain (free). C0/C1 are per-partition
column slices. The custom op carries a hand-authored 2X_1PORT uop program
(two parallel max/max/mul chains over the packed bf16 pair via SRC_*_HI,
written through WR0_LO/WR0_HI).

The output matrix is SYMMETRIC (max is), so the device only computes the
36 upper-triangle [128,128] blocks of each channel's (N,N) matrix. Row
strip r (rows r*128..r*128+127, cols r*128..N) is paired with strip 7-r
so each pair packs to a constant 9*128=1152 columns: the o2 tile is
[128, 4, 1152] = 9 KiB/partition, stored with ONE dma_start per channel
(1.125 MiB, all 16 SDMA engines) on the otherwise-idle Sync queue —
dma_start triggers cost ~0.6-0.8us of issuing-sequencer time each, which
is why the stores are batched this coarsely. The host mirrors the lower
triangle during unsharding (pure data placement, like the bf16->f32 cast).

The true diagonal d1*d2 (d1 = relu(t'1+th5_1), d2 = sigmoid(relu(t'2+th5_2))
= max(sigmoid(t'2+th5_2), 0.5)) is computed on-device as one extra custom-op
call on a [128, 8, 8] f32 tile and exported as a second tiny output; the
host scatters it onto the diagonals.

Startup is load-receipt-bound: the emb high half rides one merged dma with
the column-weight matrix, channel 0's replication weights ride a dedicated
2.5 KiB load (DMA-write deps are tile-granular — one dma per head tile),
and channel 0 runs its narrow high-half B-strips while the low half is
still in flight (its v-copy runs on the then-idle DVE). The tail is
DMA-drain-bound: the final two channels are interleaved at strip-pair
granularity with contiguous 288 KiB pair stores on alternating queues, so
their combined 2.25 MiB spreads over both compute windows and the final
drain is one pair. (Finer strip-granular stores were tried and are slower:
odd non-contiguous shapes cut SDMA efficiency; so is a quarter-granularity
channel-0 head: it shortens the head but lengthens the DVE stream more.)

Everything on-device is bf16 (harness tolerance 2e-2 >> bf16 rounding); the
host converts back to f32. This halves HBM write traffic and halves matmul
LDWEIGHTS time.

Measured on TRN2 (fast clock state): best 48.6 us, typical 48.6-51 us, vs
the 84.8 us baseline. Engine budget: DVE GATED stream 31.55 us, stable to
+-10 ns across runs (64 ops: 19.2 us at the 2X_1PORT peak of 2 elem/lane/
cycle + ~188 ns/op custom-dispatch+SBUF-roundtrip overhead; op count is
forced — each strip has distinct per-partition consts, and v3 hardware
has no spare delay chain to page them), head ~11.5-12.2 us (~7 us NEFF
boot + load receipts), tail ~4.8 us (drain + receipt + ~2.5 us teardown
barrier). All run-to-run variance is DMA-side (load receipt latency at
the head, HBM write receipt/congestion in the drain); the device also has
a ~20% DVFS throttle state on some runs.
"""

import sys
import types

import ml_dtypes
import numpy as np

B, C, N, E = 2, 64, 1024, 32
NCORES = 8
EPC = B * E // NCORES  # 8 channels per core
P = 128
NB = N // P  # 8 row blocks
H = N // 2  # matmul moving free-dim limit is 512
NQ = NB // 2  # 4 packed strip-pairs per channel
W = 9 * P  # 1152: strip r (8-r blocks) + strip 7-r (r+1 blocks)

# Engage the DVE 2X_1PORT perf mode for the custom op (bf16 packed operands).
USE_2X = True
_CACHE = {}


def _ensure_hook_shim():
    """Make trace=True safe even when antenv.axon_hooks is absent."""
    try:
        import antenv.axon_hooks  # noqa: F401
    except ImportError:
        mod = types.ModuleType("antenv.axon_hooks")
        mod.get_axon_ntff_profile_hook = lambda: None
        mod.set_axon_ntff_profile_hook = lambda h: None
        sys.modules["antenv.axon_hooks"] = mod


def _build_2x_uop(base):
    """2X_1PORT program for out = max(in0,s0)*max(in1,s1).

    In 2X mode each port read delivers a packed pair of bf16 elements; the
    low element enters via SRC_0/SRC_1, the high one via SRC_0_HI/SRC_1_HI.
    Two copies of the max/max/mul chain are placed across the 8 ALU blocks;
    results are written packed via WR0_LO / WR0_HI. Per-partition scalar
    consts are shared by both chains (both elements are in the same row).
    """
    from concourse.dve_uop import (
        AluInp, AluOp, DelayInp, ENABLE, InpSel, OutPath, OutSel, UopConfig,
    )

    u = UopConfig()
    # lane j>=1 feeds delay chain j-1 at block 0
    u.enable_input(InpSel.SRC_0, 1)      # chain 0
    u.enable_input(InpSel.CONST_0, 2)    # chain 1
    u.enable_input(InpSel.SRC_1, 3)      # chain 2
    u.enable_input(InpSel.CONST_1, 4)    # chain 3
    u.enable_input(InpSel.SRC_0_HI, 5)   # chain 4
    u.enable_input(InpSel.SRC_1_HI, 6)   # chain 5
    u.require_inp0 = ENABLE
    u.require_inp1 = ENABLE
    u.trigger = base.trigger
    u.next_uop = (0, 0, 0)
    u.repeat_count = base.repeat_count

    dp = u.datapath_config
    # low chain
    dp[0].enable_alu(AluOp.MAX, AluInp.PREV_DELAY_0, AluInp.PREV_DELAY_1)
    dp[0].pass_through_delay(1, 2, 3, 4, 5)
    dp[1].enable_alu(AluOp.MAX, AluInp.PREV_DELAY_2, AluInp.PREV_DELAY_3)
    dp[1].enable_delay_from_src(DelayInp.PREV_ALU_OUT, 0)  # max_v_lo
    dp[1].pass_through_delay(1, 3, 4, 5)
    dp[2].enable_alu(AluOp.MULTIPLY, AluInp.PREV_DELAY_0, AluInp.PREV_ALU_OUT)
    dp[2].pass_through_delay(1, 3, 4, 5)
    # high chain (consts still on chains 1 and 3)
    dp[3].enable_alu(AluOp.MAX, AluInp.PREV_DELAY_4, AluInp.PREV_DELAY_1)
    dp[3].enable_delay_from_src(DelayInp.PREV_ALU_OUT, 0)  # out_lo
    dp[3].pass_through_delay(3, 5)
    dp[4].enable_alu(AluOp.MAX, AluInp.PREV_DELAY_5, AluInp.PREV_DELAY_3)
    dp[4].enable_delay_from_src(DelayInp.PREV_ALU_OUT, 1)  # max_v_hi
    dp[4].pass_through_delay(0)
    dp[5].enable_alu(AluOp.MULTIPLY, AluInp.PREV_DELAY_1, AluInp.PREV_ALU_OUT)
    dp[5].pass_through_delay(0)
    dp[6].pass_through_alu()
    dp[6].pass_through_delay(0)
    dp[7].pass_through_alu()
    dp[7].pass_through_delay(0)

    u.out[OutPath.WR0_LO] = OutSel.DELAY_0
    u.out_enable[OutPath.WR0_LO] = ENABLE
    u.out[OutPath.WR0_HI] = OutSel.ALU_OUT
    u.out_enable[OutPath.WR0_HI] = ENABLE
    return u


def _register_gated_maxmul():
    """Register the fused out = max(in0,s0)*max(in1,s1) custom DVE op."""
    import concourse.dve_ops as dve_ops
    from concourse.dve_ops import DveOp, OPS, has_src1, _COMPILE_CACHE
    from concourse.dve_spec import C0, C1, Spec, Src0, Src1, lower, maxx
    from concourse.dve_uop import DveOpSpec

    for op in OPS:
        if op.name == "GATED_MAXMUL_ANT":
            return op

    spec = Spec(
        body=maxx(Src0, C0) * maxx(Src1, C1),
        reference=lambda in0, in1, s0, s1, imm2: np.maximum(in0, s0)
        * np.maximum(in1, s1),
    )
    op = DveOp("GATED_MAXMUL_ANT", spec, subdim=False, uops_sha={})
    OPS.append(op)
    # Rebuild the registry views that were snapshotted at import time.
    dve_ops.CUSTOM_DVE_SPECS[op.name] = op.spec
    opcode = dve_ops._CUSTOM_DVE_ROW_BASE + len(OPS) - 1
    assert opcode < 0x20
    dve_ops._SUB_OPCODE_FOR_NAME[op.name] = opcode
    # Pre-seed the compile cache with a spec that (optionally) carries the
    # perf-mode uop programs; compile() then returns it without the sha check.
    for ver in ("v3", "v4"):
        uops = lower(spec, ver=ver)
        kw = {}
        if USE_2X:
            kw = dict(uops_2x=[_build_2x_uop(uops[0])])
        s = DveOpSpec(
            name=op.name, opcode=opcode, uops=uops,
            rd1_en=has_src1(spec), **kw,
        )
        op.uops_sha[ver] = s.sha(ver)
        _COMPILE_CACHE[(op.name, ver)] = s
    return op


def _build_program():
    import concourse.bacc as bacc
    import concourse.mybir as mybir
    import concourse.tile as tile

    f32 = mybir.dt.float32
    bf16 = mybir.dt.bfloat16
    AF = mybir.ActivationFunctionType

    gated_op = _register_gated_maxmul()

    nc = bacc.Bacc("TRN2", target_bir_lowering=False, debug=False, num_devices=NCORES)

    # hd = every head-critical tensor packed together: emb cols 512..1024,
    # the [C,40] column-weight matrix, and channel 0's replicated v|g
    # weight slices. Loaded into ONE tile by TWO parallel dmas (sync +
    # scalar halves) — the head matmuls need all of it, so the
    # tile-granular DMA dependency is exactly right, and halving the
    # transfer length lands it ~0.4us earlier.
    HD = H + 40 + 2 * P
    hd = nc.declare_dram_parameter("hd", [C, HD], bf16, isOutput=False)
    emb0 = nc.declare_dram_parameter("emb0", [C, H], bf16, isOutput=False)
    wrepv = nc.declare_dram_parameter("wrepv", [C, N], bf16, isOutput=False)
    wrepg = nc.declare_dram_parameter("wrepg", [C, N], bf16, isOutput=False)
    th5bc = nc.declare_dram_parameter("th5bc", [P, NB * 2 * EPC], f32, isOutput=False)
    out = nc.declare_dram_parameter("out", [EPC, P, NQ, W], bf16, isOutput=True)
    diag = nc.declare_dram_parameter("diag", [P, NB * EPC], f32, isOutput=True)

    def custom(out_ap, in0, in1, s0, s1):
        bi = nc.vector._custom_dve(gated_op, out=out_ap, in0=in0, in1=in1, s0=s0, s1=s1)
        if USE_2X:
            bi.ins.perf_max = 1  # engine may escalate to 2X_1PORT
        return bi

    with tile.TileContext(nc, pool_alloc_mode="queue") as tc:
        with (
            tc.tile_pool(name="const", bufs=1) as cpool,
            tc.tile_pool(name="rows", bufs=1) as rpool,
        ):
            # Loads: trigger cost is ~0.6-0.8us each on the issuing
            # sequencer. Sync carries the replication dependencies (emb,
            # then w for the column matmuls); scalar carries the rest in
            # parallel while its ACT engine does the sigmoid table load.
            sb_warm = cpool.tile([1, EPC], f32)
            nc.vector.memset(sb_warm[:], 0.0)
            # Warm the ACT sigmoid table first, during the input loads.
            nc.scalar.activation(sb_warm[:], sb_warm[:], AF.Sigmoid)
            # h1 (cols 512..1024) first: the whole high-half dependency
            # chain (replication, columns r4-7, the narrow B-strips) can
            # run before the low half even lands. DMA-write deps are
            # tile-granular, so every independently-consumed load gets its
            # own tile, loaded by exactly one dma.
            # Head-critical loads on HWDGE (sync/scalar). SWDGE via the
            # GpSimd queue was tried — it clears the entry barrier ~0.9us
            # earlier, but the Q7 descriptor-generation overhead costs more
            # than that and the head LOST ~1.6us.
            HDH = HD // 2
            sb_hd = cpool.tile([C, HD], bf16)
            nc.sync.dma_start(out=sb_hd[:, 0:HDH], in_=hd[:, 0:HDH])
            nc.scalar.dma_start(out=sb_hd[:, HDH:HD], in_=hd[:, HDH:HD])
            sb_emb0 = cpool.tile([C, H], bf16)
            nc.sync.dma_start(out=sb_emb0[:], in_=emb0[:])
            sb_w0 = sb_hd[:, H + 40:HD]
            sb_wrepv = cpool.tile([C, N], bf16)
            nc.scalar.dma_start(out=sb_wrepv[:, P:N], in_=wrepv[:, P:N])
            sb_wrepg = cpool.tile([C, N], bf16)
            nc.scalar.dma_start(out=sb_wrepg[:, P:N], in_=wrepg[:, P:N])
            sb_th5bc = cpool.tile([P, NB, 2 * EPC], f32)
            nc.scalar.dma_start(out=sb_th5bc[:], in_=th5bc[:])
            sb_emb1 = sb_hd[:, 0:H]
            sb_w = sb_hd[:, H:H + 40]

            # Column-space per-partition scalars: vgc[:, r, 0:8] holds
            # max(t'1, 0) at node r*128+p, [:, r, 8:16] max(sigmoid(t'2), .5).
            sb_vgc = rpool.tile([P, NB, 2 * EPC], f32)
            sb_gs = rpool.tile([P, NB, EPC], f32)
            sb_u = rpool.tile([P, NB, 2 * EPC], f32)
            sb_s2 = rpool.tile([P, NB, EPC], f32)
            sb_pd = rpool.tile([P, NB, EPC], f32)  # true diag d1*d2

            with (
                tc.tile_pool(name="psum", bufs=3, space="PSUM") as pp,
                tc.tile_pool(name="colps", bufs=1, space="PSUM") as cp,
                tc.tile_pool(name="jrepsb", bufs=3) as jsb,
                tc.tile_pool(name="work", bufs=3) as wp,
            ):
                ps_c = cp.tile([P, NB, 40], f32, tag="ps_c")

                def repl_half(ch, h, vj, gj):
                    """Replicated-weight matmul for one emb half: ps[p, j] =
                    t'_k[ch, j] for every partition p, straight from emb.
                    The g-side sigmoid rides the ACT PSUM->SBUF drain."""
                    cs = slice(ch * P, (ch + 1) * P)
                    hs = slice(h * H, (h + 1) * H)
                    sb_embh = sb_emb1 if h == 1 else sb_emb0[:]
                    lv = sb_w0[:, 0:P] if ch == 0 else sb_wrepv[:, cs]
                    lg = sb_w0[:, P:2 * P] if ch == 0 else sb_wrepg[:, cs]
                    ps_vh = pp.tile([P, H], f32, tag="ps_vh")
                    nc.tensor.matmul(
                        ps_vh[:], lhsT=lv,
                        rhs=sb_embh, start=True, stop=True,
                    )
                    if ch == 0 and h == 1:
                        # DVE is idle during the head; its queue runs this
                        # copy while ACT starts on the g-side sigmoid.
                        nc.vector.tensor_scalar_add(vj[:, hs], ps_vh[:], 0.0)
                    else:
                        nc.scalar.copy(vj[:, hs], ps_vh[:])
                    ps_gh = pp.tile([P, H], f32, tag="ps_gh")
                    nc.tensor.matmul(
                        ps_gh[:], lhsT=lg,
                        rhs=sb_embh, start=True, stop=True,
                    )
                    nc.scalar.activation(gj[:, hs], ps_gh[:], AF.Sigmoid)

                def replicate(ch):
                    vj = jsb.tile([P, N], bf16, tag="sb_vj")
                    gj = jsb.tile([P, N], bf16, tag="sb_gj")
                    repl_half(ch, 1, vj, gj)
                    repl_half(ch, 0, vj, gj)
                    return vj, gj

                def cols_mm(h):
                    """Columns: tcol[p, r, k] = t'_k[r*128+p] via emb-block
                    matmuls + the ACT sigmoid for the g-side scalars."""
                    rs = slice(h * 4, h * 4 + 4)
                    for r in range(h * 4, h * 4 + 4):
                        lo = (r - h * 4) * P
                        lhsT = (sb_hd[:, lo:lo + P] if h == 1
                                else sb_emb0[:, lo:lo + P])
                        nc.tensor.matmul(
                            ps_c[:, r, :], lhsT=lhsT,
                            rhs=sb_w, start=True, stop=True,
                        )
                    nc.scalar.activation(sb_gs[:, rs, :], ps_c[:, rs, 32:40],
                                         AF.Sigmoid)

                def cols_dve(h):
                    """Per-partition scalars for rows h*512..h*512+511:
                    v-side max(t'1, 0); g-side max(sigmoid(t'2), 0.5)."""
                    rs = slice(h * 4, h * 4 + 4)
                    nc.vector.tensor_scalar_max(
                        sb_vgc[:, rs, 0:EPC], ps_c[:, rs, 0:EPC], 0.0,
                    )
                    nc.vector.tensor_scalar_max(
                        sb_vgc[:, rs, EPC:], sb_gs[:, rs, :], 0.5,
                    )

                def gated(o2, sb_vj, sb_gj, ch, q, part):
                    wq = (NB - q) * P
                    if part == 0:  # strip q, cols q*128..N
                        custom(
                            o2[:, q, 0:wq],
                            sb_vj[:, q * P:N], sb_gj[:, q * P:N],
                            sb_vgc[:, q, ch:ch + 1],
                            sb_vgc[:, q, EPC + ch:EPC + ch + 1],
                        )
                    else:  # strip 7-q, cols (7-q)*128..N
                        rq = NB - 1 - q
                        custom(
                            o2[:, q, wq:W],
                            sb_vj[:, rq * P:N], sb_gj[:, rq * P:N],
                            sb_vgc[:, rq, ch:ch + 1],
                            sb_vgc[:, rq, EPC + ch:EPC + ch + 1],
                        )

                # Channel 0 head: the full high-half chain (replication h1,
                # columns r4-7, scalars) is emitted before anything touches
                # the low half, so the narrow B-strips (cols >= 512 only)
                # start while h0 still flows through PE/ACT. The r0-3 DVE
                # scalar clips are emitted after the B-strips (in-order DVE
                # queue must not wait on the h0 chain before them).
                vj0 = jsb.tile([P, N], bf16, tag="sb_vj")
                gj0 = jsb.tile([P, N], bf16, tag="sb_gj")
                o2 = wp.tile([P, NQ, W], bf16, tag="o2")
                repl_half(0, 1, vj0, gj0)
                cols_mm(1)
                cols_dve(1)
                repl_half(0, 0, vj0, gj0)
                cols_mm(0)
                for q in range(NQ):
                    gated(o2, vj0, gj0, 0, q, 1)        # strips 7..4
                cols_dve(0)
                for q in range(NQ):
                    gated(o2, vj0, gj0, 0, q, 0)        # strips 0..3
                nc.sync.dma_start(out=out[0], in_=o2[:])

                # True-diagonal path, staged across the first two channel
                # windows so no engine's in-order queue ever waits across a
                # GATED batch: u = t' + th5 on DVE after ch0, sigmoid on
                # ACT + fused max*max on DVE after ch1.
                # Only the g-side u is needed early (ch1's ACT sigmoid reads
                # it); the v-side add joins the end-of-stream diag chain.
                ALU = mybir.AluOpType
                nc.vector.scalar_tensor_tensor(
                    sb_u[:, :, EPC:], ps_c[:, :, 32:40], 1.0,
                    sb_th5bc[:, :, EPC:], op0=ALU.mult, op1=ALU.add,
                )

                for ch in range(1, EPC - 2):
                    sb_vj, sb_gj = replicate(ch)

                    # Upper-triangle strips, packed in constant-width pairs:
                    # o2[:, q, 0:wq]  = strip q   (rows q*128+p, cols q*128..N)
                    # o2[:, q, wq:W]  = strip 7-q (rows (7-q)*128+p, cols (7-q)*128..N)
                    o2 = wp.tile([P, NQ, W], bf16, tag="o2")
                    for q in range(NQ):
                        gated(o2, sb_vj, sb_gj, ch, q, 0)
                        gated(o2, sb_vj, sb_gj, ch, q, 1)
                    # One 1.125 MiB store per channel on the otherwise-
                    # idle Sync queue; all 16 SDMA engines participate.
                    nc.sync.dma_start(out=out[ch], in_=o2[:])

                    if ch == 1:
                        # Diagonal g-side sigmoid rides ACT's slack here;
                        # the fused max*max and the diag store are emitted
                        # after the last GATED so they hide under the tail
                        # DMA drain instead of sitting in the DVE stream.
                        nc.scalar.activation(sb_s2[:], sb_u[:, :, EPC:],
                                             AF.Sigmoid)

                # Final two channels interleaved at PAIR granularity with
                # contiguous 288 KiB pair stores: ch6's bytes ship inside
                # its own compute window and ch7's spread over ~2 windows,
                # so the post-compute DMA drain is one pair, not 2.25 MiB.
                # (Strip-granular stores were tried and are slower: the
                # odd non-contiguous shapes cut SDMA efficiency.)
                c6, c7 = EPC - 2, EPC - 1
                vj6, gj6 = replicate(c6)
                vj7, gj7 = replicate(c7)
                o6 = wp.tile([P, NQ, W], bf16, tag="o2")
                o7 = wp.tile([P, NQ, W], bf16, tag="o2")
                nst = 0

                def pair_store(ch, o2, q):
                    nonlocal nst
                    eng = nc.sync if nst % 2 == 0 else nc.scalar
                    nst += 1
                    eng.dma_start(out=out[ch, :, q:q + 1, :],
                                  in_=o2[:, q:q + 1, :])

                # ch6 pairs 0,1 first (repl7's ACT copies finish ~1.5us into
                # this window), then alternate, ch7's last pair last.
                order = [(c6, o6, vj6, gj6, 0), (c6, o6, vj6, gj6, 1),
                         (c7, o7, vj7, gj7, 0), (c6, o6, vj6, gj6, 2),
                         (c7, o7, vj7, gj7, 1), (c6, o6, vj6, gj6, 3),
                         (c7, o7, vj7, gj7, 2), (c7, o7, vj7, gj7, 3)]
                for ch, o2, vj, gj, q in order:
                    gated(o2, vj, gj, ch, q, 0)
                    gated(o2, vj, gj, ch, q, 1)
                    pair_store(ch, o2, q)

                # Diagonal d1*d2 = max(u1,0)*max(sigmoid(u2),0.5): the u1
                # add and one custom-DVE op with immediate scalars, run
                # concurrently with the final pair's DMA drain; the 32 KiB
                # diag store's receipt lands with the last pair store's.
                nc.vector.scalar_tensor_tensor(
                    sb_u[:, :, 0:EPC], ps_c[:, :, 0:EPC], 1.0,
                    sb_th5bc[:, :, 0:EPC], op0=ALU.mult, op1=ALU.add,
                )
                custom(sb_pd[:], sb_u[:, :, 0:EPC], sb_s2[:], 0.0, 0.5)
                nc.sync.dma_start(out=diag[:], in_=sb_pd[:])

    nc.compile()
    return nc


def _get_program():
    if "nc" not in _CACHE:
        _CACHE["nc"] = _build_program()
    return _CACHE["nc"]


def kernel(**inputs):
    _ensure_hook_shim()
    from concourse.bass_utils import run_bass_kernel_spmd

    bf = ml_dtypes.bfloat16
    emb = np.ascontiguousarray(np.asarray(inputs["emb"], dtype=np.float32)).astype(bf)
    th12_1 = np.asarray(inputs["th12_1"], dtype=np.float32)
    th12_2 = np.asarray(inputs["th12_2"], dtype=np.float32)
    th5_1 = np.asarray(inputs["th5_1"], dtype=np.float32)
    th5_2 = np.asarray(inputs["th5_2"], dtype=np.float32)

    in_maps = []
    for k in range(NCORES):
        b = k // (NCORES // B)
        e0 = (k % (NCORES // B)) * EPC
        # The 2x of the reference's "m + m" is folded into the weights.
        w2_1 = (2.0 * th12_1[e0:e0 + EPC]).astype(bf)  # [EPC, C]
        w2_2 = (2.0 * th12_2[e0:e0 + EPC]).astype(bf)
        wm = np.zeros((C, 40), dtype=bf)
        wm[:, 0:EPC] = w2_1.T
        wm[:, 32:40] = w2_2.T
        # wrep[c, ch*128+m] = 2*th12[e0+ch, c] for all m (replicated cols).
        wrepv = np.repeat(w2_1.T[:, :, None], P, axis=2).reshape(C, N)
        wrepg = np.repeat(w2_2.T[:, :, None], P, axis=2).reshape(C, N)
        th5cat = np.concatenate([th5_1[e0:e0 + EPC], th5_2[e0:e0 + EPC]])  # [16]
        th5bc = np.tile(th5cat[None, :], (P, NB)).astype(np.float32)  # [128, 128]
        in_maps.append(
            {
                "hd": np.ascontiguousarray(
                    np.concatenate([emb[b][:, H:N], wm,
                                    wrepv[:, 0:P], wrepg[:, 0:P]], axis=1)),
                "emb0": np.ascontiguousarray(emb[b][:, 0:H]),
                "wrepv": np.ascontiguousarray(wrepv),
                "wrepg": np.ascontiguousarray(wrepg),
                "th5bc": th5bc,
            }
        )

    nc = _get_program()
    res = run_bass_kernel_spmd(nc, in_maps, core_ids=list(range(NCORES)))
    _CACHE["last_result"] = res

    out = np.empty((B, E, N, N), dtype=np.float32)
    for k in range(NCORES):
        b = k // (NCORES // B)
        e0 = (k % (NCORES // B)) * EPC
        a = np.asarray(res.results[k]["out"], dtype=np.float32)  # [EPC,P,NQ,W]
        dg = np.asarray(res.results[k]["diag"], dtype=np.float32)  # [P, NB*EPC]
        oc = out[b, e0:e0 + EPC]  # [EPC, N, N] view
        # Unpack the paired upper-triangle strips.
        for q in range(NQ):
            wq = (NB - q) * P
            rq = NB - 1 - q
            oc[:, q * P:(q + 1) * P, q * P:] = a[:, :, q, 0:wq]
            oc[:, rq * P:(rq + 1) * P, rq * P:] = a[:, :, q, wq:W]
        # Mirror the lower triangle (the matrix is symmetric).
        o6 = oc.reshape(EPC, NB, P, NB, P)
        for r in range(NB):
            for c in range(r):
                o6[:, r, :, c, :] = o6[:, c, :, r, :].transpose(0, 2, 1)
        # Place the true diagonal (f32, exact).
        dv = dg.reshape(P, NB, EPC).transpose(2, 1, 0).reshape(EPC, N)
        oc.reshape(EPC, N * N)[:, ::N + 1] = dv
    return out
